# revision 83
# baseline (speedup 1.0000x reference)
"""Trainium2 Bass kernel for nn_DiffPoolPrompt (GCN conv + softmax pooling prompt).

Reference computation:
    h = x + sum(cluster_emb, 0)
    logits = GCNConv(h, W, bias, edge_index)   # sym-normalized, self-loops
    s = softmax(logits, axis=1)
    out = x + s @ cluster_emb

Distribution strategy (8 NeuronCores):
  - Nodes sharded contiguously: core c owns nodes [c*12500, (c+1)*12500).
  - Each core computes g = dinv * (x @ W + cW) for its nodes from a
    host-pre-transposed bf16 x (no on-device transposes), accumulating each
    window-quarter of rank-rows in one PSUM tile. The g table is written
    chunk-major (row u = r*128 + p), so each quarter forms one of 4
    collective chunks: as soon as phase B finishes a quarter, its compact
    bf16 slice all-gathers (64KB in / 512KB out), overlapping the rest of
    phase B and earlier windows' gathers.
  - Every window's compact [25600, 10] bf16 table is expanded to a
    256B-strided gather table by one strided DRAM->DRAM DMA per window
    (pad columns are never initialized or read).
  - Edges partitioned by destination core. Each destination's sources are
    assigned windows by a host-side greedy balancer (per-dst counts near
    ceil(deg/4)), and rank-rows group nodes with equal max-per-window counts,
    cutting gather padding to ~1.25x. Windowed indirect DMA gathers (12K
    tokens per instruction, idx tables on 32 partitions) fetch 10-dim bf16
    messages; merged strided vector reduces accumulate f32 per node.
  - The reference's added self-loop is folded in as a direct vector add of
    the local g tile (no gather tokens).
  - softmax + s @ emb + x run in transposed (channel-major) layout; the last
    window is gathered in 4 row segments with each segment's softmax emitted
    immediately and output groups trailing two segments behind, so the tail
    overlaps the final gathers; host un-transposes and un-permutes rows of
    the bf16 output.
  - Host work is index-only plus data movement (permute/transpose/cast) of
    x and the output.
"""

import numpy as np

import concourse.bass as bass
import concourse.bacc as bacc
import concourse.tile as tile
import concourse.mybir as mybir
from concourse.bass_utils import run_bass_kernel_spmd
from concourse.masks import make_identity
from concourse import ap_utils

N_NODES = 100000
N_EDGES = 3200000
IN_CH = 256
K = 10
NCORES = 8
P = 128
NPC = N_NODES // NCORES          # 12500 nodes per core
R = 100                          # rank-row chunks per core
NPADJ = P * R                    # 12800 padded slots per core
NWIN = 4                         # gather windows (int16 idx limit)
# window sizes in rank-rows (equal sizes measured best; the framework
# supports unequal windows)
WROWS = [25, 25, 25, 25]
WOFF = [0, 25, 50, 75]           # first rank-row of each window
QN_W = [w * P for w in WROWS]    # per-core slots per window
ZSLOT_W = [(w - 1) * P + 96 for w in WROWS]   # zero row (partition 96)
QCAP_W = ZSLOT_W                 # real sources max per (core, window)
WINR_W = [NCORES * q for q in QN_W]           # table rows per window
EOFF = [0, 25600, 51200, 76800]  # window base row in the gather table
TSTRIDE = 128                    # expanded table row stride in bf16 (256B)
GCH = 12288                      # gather tokens per instruction (ring <=16336)

F32 = mybir.dt.float32
BF16 = mybir.dt.bfloat16

# ----------------------------------------------------------------------------
# raw dma_gather (bass.dma_gather minus the 256B-elem assert; elem=20B works)
# ----------------------------------------------------------------------------


def _raw_dma_gather(gp, out_ap, in_ap, idxs_ap, num_idxs, elem_size, elem_step,
                    single_packet=False, queue_num=0):
    assert idxs_ap.dtype == mybir.dt.int16
    assert in_ap.space == bass.MemorySpace.DRAM
    assert idxs_ap.space == bass.MemorySpace.SBUF
    assert out_ap.space == bass.MemorySpace.SBUF
    assert in_ap.dtype == out_ap.dtype
    assert ap_utils.ap_is_contiguous(in_ap.ap[1:])
    assert ap_utils.ap_is_contiguous(out_ap.ap[1:])
    assert ap_utils.ap_is_contiguous(idxs_ap.ap[1:])
    assert in_ap.ap[-1][1] == elem_size and out_ap.ap[-1][1] == elem_size
    assert in_ap.ap[0][0] == elem_step
    stride_bytes = elem_step * mybir.dt.size(in_ap.dtype)
    stride_bytes_256 = stride_bytes // 256
    assert stride_bytes_256 * 256 == stride_bytes and 0 < stride_bytes_256 < 256
    _in_ap = gp.lower_ap_dma(in_ap, for_custom_bir_dma=True)
    _idxs_ap = gp.lower_ap(idxs_ap)
    _out_ap = gp.lower_ap(out_ap)
    return gp.add_instruction(
        mybir.InstDMAGatherAnt(
            name=gp.bass.get_next_instruction_name(),
            ins=[*_in_ap, _idxs_ap, gp.lower_val_access(gp.to_reg(num_idxs))],
            outs=[_out_ap],
            transpose=False,
            num_idxs=num_idxs,
            elem_size=elem_size,
            stride_bytes_256=stride_bytes_256,
            gen_mode=0,
            single_packet=single_packet,
            queue_num=queue_num,
            sbuf_tokens_per_rank=0,
            sbuf_free_dim_per_rank=0,
            sbuf_free_dim_pad_per_rank=0,
            sbuf_byte_offset=0,
        )
    )


# ----------------------------------------------------------------------------
# Host-side sharding / index prep (numpy, index-only)
# ----------------------------------------------------------------------------


def _greedy_windows(src, dst):
    """Assign every source node a window in [0,4) so that each destination's
    in-edges split ~evenly across windows (count <= ceil(deg/4) + small),
    respecting per-(core, quarter) capacity QCAP. Returns (qa, counts)."""
    out_deg = np.bincount(src, minlength=N_NODES)
    in_deg = np.bincount(dst, minlength=N_NODES)
    # per-dst per-window target, proportional to window size
    wsh = np.asarray(WROWS, dtype=np.int64)
    t_dw = -(-(in_deg[:, None] * wsh[None, :]) // R)   # ceil(d * rows_w / R)
    qcap = np.asarray(QCAP_W, dtype=np.int64)
    order_e = np.argsort(src, kind="stable")
    d_sorted = dst[order_e]
    indptr = np.concatenate([[0], np.cumsum(out_deg)])

    src_order = np.argsort(-out_deg, kind="stable")
    c = np.zeros((N_NODES, NWIN), dtype=np.int64)
    ncap = np.zeros((NCORES, NWIN), dtype=np.int64)
    qa = np.full(N_NODES, -1, dtype=np.int64)
    PEN = 64                                       # defect penalty weight
    CH = 256

    def assign_chunk(nodes, refine):
        cnts = out_deg[nodes]
        starts = indptr[nodes]
        total = int(cnts.sum())
        if total > 0:
            ends = np.cumsum(cnts)
            pos = (np.arange(total)
                   - np.repeat(ends - cnts, cnts)
                   + np.repeat(starts, cnts))
            dd = d_sorted[pos]
        else:
            dd = np.empty(0, dtype=np.int64)
        seg = np.repeat(np.arange(len(nodes)), cnts)
        if refine:
            wold = np.repeat(qa[nodes], cnts)
            np.add.at(c, (dd, wold), -1)
            np.add.at(ncap, (nodes // NPC, qa[nodes]), -1)
        cd = c[dd]                                 # [E, 4]
        td = t_dw[dd]
        f = np.where(cd >= td, (cd - td + 1) * PEN,
                     (cd * R) // (NWIN * wsh[None, :]))
        sc = np.zeros((len(nodes), NWIN), dtype=np.int64)
        np.add.at(sc, seg, f)
        cidx = nodes // NPC
        sc = sc.astype(np.float64)
        sc[ncap[cidx] >= qcap[None, :]] = np.inf
        w = np.argmin(sc, axis=1).astype(np.int64)
        np.add.at(ncap, (cidx, w), 1)
        over_mask = ncap[cidx, w] > qcap[w]
        if over_mask.any():
            for j in np.where(over_mask)[0]:
                cj, wj = cidx[j], w[j]
                if ncap[cj, wj] <= qcap[wj]:
                    continue
                ncap[cj, wj] -= 1
                alt_sc = np.where(ncap[cj] < qcap,
                                  np.nan_to_num(sc[j, :], posinf=1e17), np.inf)
                alt = int(np.argmin(alt_sc))
                w[j] = alt
                ncap[cj, alt] += 1
        qa[nodes] = w
        if total > 0:
            np.add.at(c, (dd, np.repeat(w, cnts)), 1)

    for i0 in range(0, N_NODES, CH):
        assign_chunk(src_order[i0:i0 + CH], refine=False)
    for _ in range(2):
        for i0 in range(0, N_NODES, CH):
            assign_chunk(src_order[i0:i0 + CH], refine=True)
    assert (ncap <= qcap[None, :]).all() and (qa >= 0).all()
    return qa.astype(np.int8), c


def host_prep(edge_index):
    """Partition + sort edges, build per-core windowed gather plans."""
    src = np.asarray(edge_index[0], dtype=np.int64)
    dst = np.asarray(edge_index[1], dtype=np.int64)

    deg_all = np.bincount(dst, minlength=N_NODES).astype(np.int64) + 1

    qa, cnt_dw = _greedy_windows(src, dst)
    qa = qa.astype(np.int64)
    # row-grouping key: nodes with similar max-per-window counts share a row
    mkey = cnt_dw.max(axis=1) * 1000 + deg_all

    # per-core slot assignment: within (core, quarter), sort by mkey desc;
    # i-th node -> local (rq = i//P, p = i%P), global r = q*QR + rq
    p_of = np.empty(N_NODES, dtype=np.int64)
    r_of = np.empty(N_NODES, dtype=np.int64)
    node_at = np.full((NCORES, P, R), -1, dtype=np.int64)
    for cc in range(NCORES):
        base = cc * NPC
        loc = np.arange(base, base + NPC)
        for q in range(NWIN):
            nq = loc[qa[loc] == q]
            nq = nq[np.argsort(-mkey[nq], kind="stable")]
            i = np.arange(len(nq))
            rr = WOFF[q] + i // P
            pp = i % P
            p_of[nq] = pp
            r_of[nq] = rr
            node_at[cc, pp, rr] = nq
    # window-local table row: v = (r - WOFF[w])*P + p, row = core*QN_W[w] + v
    woff_of = np.asarray(WOFF, dtype=np.int64)[qa]
    qn_of = np.asarray(QN_W, dtype=np.int64)[qa]
    v_of = (r_of - woff_of) * P + p_of
    tbl = (np.arange(N_NODES) // NPC) * qn_of + v_of

    # per-core edge streams (dst-partitioned; NO self-loops appended)
    K_w = np.zeros((NWIN, R), dtype=np.int64)
    percore = []
    for cc in range(NCORES):
        lo, hi = cc * NPC, (cc + 1) * NPC
        m = (dst >= lo) & (dst < hi)
        e_p = p_of[dst[m]]
        e_r = r_of[dst[m]]
        e_w = qa[src[m]]
        e_t = tbl[src[m]]
        percore.append((e_p, e_r, e_w, e_t))
        key = (e_w * R + e_r) * P + e_p
        cnt = np.bincount(key, minlength=NWIN * R * P).reshape(NWIN, R, P)
        K_w = np.maximum(K_w, cnt.max(axis=2))

    Koff_w = np.cumsum(
        np.concatenate([np.zeros((NWIN, 1), np.int64), K_w], 1), 1)[:, :-1]
    SK_w = K_w.sum(axis=1)
    T_w = SK_w * P
    tok_off = np.concatenate([[0], np.cumsum(T_w)])
    TOT16 = int(tok_off[-1]) // 16

    idxs = []
    degs = []
    for cc in range(NCORES):
        e_p, e_r, e_w, e_t = percore[cc]
        order = np.lexsort((e_p, e_r, e_w))
        e_p, e_r, e_w, e_t = e_p[order], e_r[order], e_w[order], e_t[order]
        key = (e_w * R + e_r) * P + e_p
        node_counts = np.bincount(key, minlength=NWIN * R * P)
        k_within = np.arange(len(key)) - np.repeat(
            np.concatenate([[0], np.cumsum(node_counts)])[:-1], node_counts)
        t = tok_off[e_w] + (Koff_w[e_w, e_r] + k_within) * P + e_p
        # pads point at this core's per-window zero row (slot v = ZSLOT_W[w])
        flat = np.empty(int(tok_off[-1]), dtype=np.int16)
        for w in range(NWIN):
            flat[tok_off[w]:tok_off[w + 1]] = cc * QN_W[w] + ZSLOT_W[w]
        flat[t] = e_t.astype(np.int16)
        wrap = flat.reshape(TOT16, 16).T              # [16, TOT16]
        idxs.append(np.ascontiguousarray(
            np.concatenate([wrap, wrap], axis=0)))    # 2 replicas: [32, TOT16]

        dg = np.ones((P, R), dtype=np.float32)
        real = node_at[cc] >= 0
        dg[real] = deg_all[node_at[cc][real]].astype(np.float32)
        degs.append(dg)

    return {"node_at": node_at, "deg": degs, "idx": idxs,
            "K_w": K_w, "Koff_w": Koff_w, "tok_off": tok_off, "TOT16": TOT16}


# ----------------------------------------------------------------------------
# Device kernel
# ----------------------------------------------------------------------------

_BUILD_CACHE = {}


def build_kernel(K_w, TOT16):
    K_w = np.asarray(K_w, dtype=np.int64)
    key = (TOT16,) + tuple(int(k) for k in K_w.ravel())
    if key in _BUILD_CACHE:
        return _BUILD_CACHE[key]
    Koff_w = np.cumsum(
        np.concatenate([np.zeros((NWIN, 1), np.int64), K_w], 1), 1)[:, :-1]
    SK_w = K_w.sum(axis=1)
    tok_off = np.concatenate([[0], np.cumsum(SK_w * P)])

    nc = bacc.Bacc("TRN2", target_bir_lowering=False, debug=False,
                   num_devices=NCORES)

    x_in = nc.dram_tensor("x", [P, 2 * NPADJ], BF16, kind="ExternalInput").ap()
    w_in = nc.dram_tensor("w", [IN_CH, K], F32, kind="ExternalInput").ap()
    bias_in = nc.dram_tensor("bias", [1, K], F32, kind="ExternalInput").ap()
    emb_in = nc.dram_tensor("emb", [K, IN_CH], F32, kind="ExternalInput").ap()
    deg_in = nc.dram_tensor("deg", [P, R], F32, kind="ExternalInput").ap()
    idx_in = nc.dram_tensor("idx", [32, TOT16], mybir.dt.int16,
                            kind="ExternalInput").ap()
    out = nc.dram_tensor("out", [P, 2 * NPADJ], BF16,
                         kind="ExternalOutput").ap()

    GCOLS = GCH // P               # max gather columns per instruction

    with tile.TileContext(nc) as tc:
        with tc.tile_pool(name="big", bufs=1) as big, \
             tc.tile_pool(name="small", bufs=1) as small, \
             tc.tile_pool(name="msg", bufs=8) as msgp, \
             tc.tile_pool(name="idx16", bufs=12) as idxp, \
             tc.tile_pool(name="ops", bufs=3) as opsp, \
             tc.tile_pool(name="ps0", bufs=1, space="PSUM") as ps0, \
             tc.tile_pool(name="psHW", bufs=1, space="PSUM") as psHW, \
             tc.tile_pool(name="psST", bufs=2, space="PSUM") as psST, \
             tc.tile_pool(name="psP", bufs=2, space="PSUM") as psP, \
             tc.tile_pool(name="dram", bufs=1, space="DRAM") as dram:

            # ---- resident loads (small tensors first; x split per quarter so
            # phase B / collectives start as early as possible)
            w_sb = small.tile([P, 2 * K], F32)            # [ch%128, 2 chunks]
            nc.sync.dma_start(w_sb[:, 0:K], w_in[0:P, :])
            nc.sync.dma_start(w_sb[:, K:2 * K], w_in[P:2 * P, :])
            w16_sb = small.tile([P, 2 * K], BF16)
            nc.vector.tensor_copy(w16_sb[:], w_sb[:])
            emb_sb = small.tile([K, IN_CH], F32)
            nc.sync.dma_start(emb_sb[:], emb_in[:])
            emb16_sb = small.tile([K, IN_CH], BF16)
            nc.vector.tensor_copy(emb16_sb[:], emb_sb[:])
            deg_sb = small.tile([P, R], F32)
            nc.sync.dma_start(deg_sb[:], deg_in[:])
            bias_sb = small.tile([1, K], F32)
            nc.sync.dma_start(bias_sb[:], bias_in[:])
            xT_sb = big.tile([P, 2 * NPADJ], BF16)        # 50KB/part
            for q in range(NWIN):
                for h in range(2):
                    cs = slice(h * NPADJ + WOFF[q] * P,
                               h * NPADJ + (WOFF[q] + WROWS[q]) * P)
                    nc.sync.dma_start(xT_sb[:, cs], x_in[:, cs])

            ident = small.tile([P, P], F32)
            make_identity(nc, ident[:])

            ones_row = small.tile([1, P], F32)
            nc.vector.memset(ones_row[:], 1.0)
            ones_col10 = small.tile([K, 1], F32)
            nc.vector.memset(ones_col10[:], 1.0)

            # ---- dinv = 1/sqrt(deg)
            dinv_sb = small.tile([P, R], F32)
            nc.scalar.activation(dinv_sb[:], deg_sb[:],
                                 mybir.ActivationFunctionType.Sqrt)
            nc.vector.reciprocal(dinv_sb[:], dinv_sb[:])

            # ---- cW = (sum_k emb[k]) @ W  as [1, 10]
            csumT_ps = ps0.tile([P, 2], F32, space="PSUM", tag="t0")
            for h in range(2):
                nc.tensor.matmul(csumT_ps[:, h:h + 1],
                                 lhsT=emb_sb[:, h * P:(h + 1) * P],
                                 rhs=ones_col10[:], start=True, stop=True)
            csumT = small.tile([P, 2], F32)
            nc.vector.tensor_copy(csumT[:], csumT_ps[:])
            cw_ps = ps0.tile([1, K], F32, space="PSUM", tag="t0")
            for h in range(2):
                nc.tensor.matmul(cw_ps[:], lhsT=csumT[:, h:h + 1],
                                 rhs=w_sb[:, h * K:(h + 1) * K],
                                 start=(h == 0), stop=(h == 1))
            cw_sb = small.tile([1, K], F32)
            nc.vector.tensor_copy(cw_sb[:], cw_ps[:])

            # bias broadcast to all partitions: [128, 10]
            biasb_ps = ps0.tile([P, K], F32, space="PSUM", tag="t0")
            nc.tensor.matmul(biasb_ps[:], lhsT=ones_row[:], rhs=bias_sb[:],
                             start=True, stop=True)
            biasb = small.tile([P, K], F32)
            nc.vector.tensor_copy(biasb[:], biasb_ps[:])

            # ---- phase B (per quarter) + bounce + chunked AllGather that
            # writes the 256B-strided gather table directly (cols 10:128 of
            # each row are never read or written)
            g_sb = big.tile([P, R * K], F32)
            g16_sb = big.tile([P, R * K], BF16)
            g_bounce = dram.tile([NPADJ, K], BF16)
            g_cmp = dram.tile([NCORES * NPADJ, K], BF16)
            g_exp = dram.tile([NCORES * NPADJ, TSTRIDE], BF16)

            boff = np.concatenate([[0], np.cumsum(np.array(QN_W))])
            for q in range(NWIN):
                qr = WROWS[q]
                hw_ps = psHW.tile([P, 31 * K], F32, space="PSUM", tag="hw",
                                  name="hw_ps")
                for rq in range(qr):
                    r = WOFF[q] + rq
                    cols = slice(rq * K, (rq + 1) * K)
                    for h in range(2):
                        nc.tensor.matmul(
                            hw_ps[:, cols],
                            lhsT=xT_sb[:, h * NPADJ + r * P:
                                       h * NPADJ + (r + 1) * P],
                            rhs=w16_sb[:, h * K:(h + 1) * K],
                            start=(h == 0), stop=False)
                    nc.tensor.matmul(hw_ps[:, cols], lhsT=ones_row[:],
                                     rhs=cw_sb[:], start=False, stop=True)
                qs = slice(WOFF[q] * K, (WOFF[q] + qr) * K)
                nc.vector.tensor_tensor(
                    out=g_sb[:, qs].rearrange("p (r j) -> p r j", j=K),
                    in0=hw_ps[:, 0:qr * K].rearrange("p (r j) -> p r j", j=K),
                    in1=dinv_sb[:, WOFF[q]:WOFF[q] + qr].unsqueeze(
                        2).to_broadcast([P, qr, K]),
                    op=mybir.AluOpType.mult)
                nc.vector.tensor_copy(g16_sb[:, qs], g_sb[:, qs])
                # force-zero row: local slot v = ZSLOT_W[q] (partition 96)
                nc.vector.memset(
                    g16_sb[96:97, (WOFF[q] + qr - 1) * K:(WOFF[q] + qr) * K],
                    0.0)
                # bounce rows u = (r - WOFF[q])*P + p (chunk-major)
                nc.sync.dma_start(
                    g_bounce[boff[q]:boff[q + 1], :].rearrange(
                        "(r p) s -> p r s", p=P),
                    g16_sb[:, qs].rearrange("p (r s) -> p r s", s=K))
                nc.gpsimd.collective_compute(
                    "AllGather", mybir.AluOpType.bypass,
                    replica_groups=[list(range(NCORES))],
                    ins=[g_bounce[boff[q]:boff[q + 1], :].opt()],
                    outs=[g_cmp[EOFF[q]:EOFF[q] + WINR_W[q], :].opt()],
                )
                # compact window -> 256B-strided gather table (cols 10:128
                # never read); single strided DRAM->DRAM copy on the
                # Activation HWDGE so SP stays free for idx prefetch
                nc.scalar.dma_start(
                    g_exp[EOFF[q]:EOFF[q] + WINR_W[q], :].rearrange(
                        "(p n) s -> p n s", p=P)[:, :, 0:K],
                    g_cmp[EOFF[q]:EOFF[q] + WINR_W[q], :].rearrange(
                        "(p n) s -> p n s", p=P))

            # ---- windowed gathers + per-(window,row) partial reduces
            partials = [big.tile([P, R * K], F32, tag=f"part{w}",
                                 name=f"part{w}")
                        for w in range(NWIN)]
            for w in range(NWIN):
                nc.vector.memset(partials[w][:], 0.0)

            agg_sb = big.tile([P, R * K], F32)
            lg = big.tile([P, R * K], F32)
            den = small.tile([P, R], F32)
            out3 = out.rearrange("p (h n) -> p h n", h=2)
            G4 = 4

            def emit_window_rows(w, r_lo, r_hi):
                win_ap = g_exp[EOFF[w]:EOFF[w] + WINR_W[w], 0:K]
                r = r_lo
                while r < r_hi:
                    kw = int(K_w[w, r])
                    if kw == 0:
                        r += 1
                        continue
                    assert kw <= GCOLS, f"slot count {kw} exceeds {GCOLS}"
                    rows = [r]
                    cols = kw
                    r2 = r + 1
                    while r2 < r_hi and cols + int(K_w[w, r2]) <= GCOLS:
                        if int(K_w[w, r2]) == 0:
                            r2 += 1
                            continue
                        rows.append(r2)
                        cols += int(K_w[w, r2])
                        r2 += 1
                    n = cols * P
                    tok0 = int(tok_off[w]) + int(Koff_w[w, rows[0]]) * P
                    msg = msgp.tile([P, GCOLS * K], BF16, tag="msgbuf",
                                    name="msg")
                    i16 = idxp.tile([32, GCH // 16], mybir.dt.int16,
                                    tag="i16", name="i16")
                    nc.sync.dma_start(i16[:, 0:n // 16],
                                      idx_in[:, tok0 // 16:(tok0 + n) // 16])
                    _raw_dma_gather(
                        nc.gpsimd,
                        msg[:, 0:cols * K].rearrange("p (c j) -> p c j", j=K),
                        win_ap, i16[:, 0:n // 16], n, K, TSTRIDE,
                        single_packet=False)
                    # merge consecutive rows with equal slot counts into one
                    # 4-D reduce (rows in a pack are consecutive, so outputs
                    # stay contiguous)
                    off = 0
                    i0 = 0
                    while i0 < len(rows):
                        kk = int(K_w[w, rows[i0]])
                        m = 1
                        while (i0 + m < len(rows)
                               and rows[i0 + m] == rows[i0] + m
                               and int(K_w[w, rows[i0 + m]]) == kk):
                            m += 1
                        r0 = rows[i0]
                        if m == 1:
                            nc.vector.tensor_reduce(
                                out=partials[w][:, r0 * K:(r0 + 1) * K],
                                in_=msg[:, off * K:(off + kk) * K].rearrange(
                                    "p (k j) -> p j k", j=K),
                                axis=mybir.AxisListType.X,
                                op=mybir.AluOpType.add)
                        else:
                            nc.vector.tensor_reduce(
                                out=partials[w][:, r0 * K:(r0 + m) * K]
                                .rearrange("p (m j) -> p m j", j=K),
                                in_=msg[:, off * K:(off + m * kk) * K]
                                .rearrange("p (m k j) -> p m j k", k=kk, j=K),
                                axis=mybir.AxisListType.X,
                                op=mybir.AluOpType.add)
                        off += m * kk
                        i0 += m
                    r = r2

            def emit_tail_softmax(r_lo, r_hi):
                RL = r_hi - r_lo
                sl = slice(r_lo * K, r_hi * K)
                # combine partials + self-loop term (g_sb itself)
                nc.vector.tensor_add(out=agg_sb[:, sl], in0=partials[0][:, sl],
                                     in1=partials[1][:, sl])
                nc.vector.tensor_add(out=agg_sb[:, sl], in0=agg_sb[:, sl],
                                     in1=partials[2][:, sl])
                nc.vector.tensor_add(out=agg_sb[:, sl], in0=agg_sb[:, sl],
                                     in1=partials[3][:, sl])
                nc.vector.tensor_add(out=agg_sb[:, sl], in0=agg_sb[:, sl],
                                     in1=g_sb[:, sl])
                # logits = dinv*agg + bias ; softmax
                lg3 = lg[:, sl].rearrange("p (r j) -> p r j", j=K)
                nc.vector.tensor_tensor(
                    out=lg3,
                    in0=agg_sb[:, sl].rearrange("p (r j) -> p r j", j=K),
                    in1=dinv_sb[:, r_lo:r_hi].unsqueeze(2).to_broadcast(
                        [P, RL, K]),
                    op=mybir.AluOpType.mult)
                nc.vector.tensor_tensor(
                    out=lg3, in0=lg3,
                    in1=biasb[:].unsqueeze(1).to_broadcast([P, RL, K]),
                    op=mybir.AluOpType.add)
                nc.scalar.activation(lg[:, sl], lg[:, sl],
                                     mybir.ActivationFunctionType.Exp)
                nc.vector.tensor_reduce(out=den[:, r_lo:r_hi], in_=lg3,
                                        axis=mybir.AxisListType.X,
                                        op=mybir.AluOpType.add)
                nc.vector.reciprocal(den[:, r_lo:r_hi], den[:, r_lo:r_hi])
                nc.vector.tensor_tensor(
                    out=lg3, in0=lg3,
                    in1=den[:, r_lo:r_hi].unsqueeze(2).to_broadcast([P, RL, K]),
                    op=mybir.AluOpType.mult)

            def emit_tail_out(r_lo, r_hi):
                # outT = xT + emb^T @ s^T  (groups of G4 rank-rows; the s^T
                # copy runs on the idle Activation engine and the +x add is
                # one 3-D op so DVE per-instruction overhead stays small)
                for g0 in range(r_lo, r_hi, G4):
                    gn = min(G4, r_hi - g0)
                    st_ps = psST.tile([K, G4 * P], F32, space="PSUM",
                                      tag="stps", name="st_ps")
                    for i in range(gn):
                        r = g0 + i
                        nc.tensor.transpose(st_ps[:, i * P:(i + 1) * P],
                                            lg[:, r * K:(r + 1) * K],
                                            ident[:])
                    st4 = opsp.tile([K, G4 * P], BF16, tag="st", name="st4")
                    nc.scalar.activation(st4[:, 0:gn * P], st_ps[:, 0:gn * P],
                                         mybir.ActivationFunctionType.Copy)
                    pp = psP.tile([P, 2 * G4 * P], F32, space="PSUM",
                                  tag="pp", name="pp")
                    for h in range(2):
                        nc.tensor.matmul(
                            pp[:, h * G4 * P:h * G4 * P + gn * P],
                            lhsT=emb16_sb[:, h * P:(h + 1) * P],
                            rhs=st4[:, 0:gn * P], start=True, stop=True)
                    out_t = opsp.tile([P, 2 * G4 * P], BF16, tag="outt",
                                      name="out_t")
                    nc.vector.tensor_tensor(
                        out=out_t[:].rearrange(
                            "p (h n) -> p h n", h=2)[:, :, 0:gn * P],
                        in0=pp[:].rearrange(
                            "p (h n) -> p h n", h=2)[:, :, 0:gn * P],
                        in1=xT_sb[:].rearrange(
                            "p (h n) -> p h n", h=2)[:, :,
                                                     g0 * P:(g0 + gn) * P],
                        op=mybir.AluOpType.add)
                    nc.sync.dma_start(
                        out3[:, :, g0 * P:(g0 + gn) * P],
                        out_t[:].rearrange("p (h n) -> p h n", h=2)[:, :,
                                                                    0:gn * P])

            # last window processed in segments; each segment's softmax is
            # emitted immediately (small DVE ops) while the heavyweight
            # output groups trail two segments behind, so later segments'
            # reduces never queue behind them on DVE
            SEGS = [(0, 50), (50, 76), (76, 92), (92, R)]
            for w in range(NWIN - 1):
                emit_window_rows(w, 0, R)
            for i, (lo, hi) in enumerate(SEGS):
                emit_window_rows(NWIN - 1, lo, hi)
                if i >= 2:
                    emit_tail_out(*SEGS[i - 2])
                emit_tail_softmax(lo, hi)
            for lo, hi in SEGS[-2:]:
                emit_tail_out(lo, hi)

    nc.compile()
    _BUILD_CACHE[key] = nc
    return nc


# ----------------------------------------------------------------------------
# Entry point
# ----------------------------------------------------------------------------


def kernel(x, edge_index, batch, W, bias, cluster_emb):
    x = np.asarray(x, dtype=np.float32)
    W = np.asarray(W, dtype=np.float32)
    bias = np.asarray(bias, dtype=np.float32).reshape(1, K)
    cluster_emb = np.asarray(cluster_emb, dtype=np.float32)

    plan = host_prep(edge_index)
    nc = build_kernel(plan["K_w"], plan["TOT16"])

    bf16np = mybir.dt.np(BF16)
    in_maps = []
    for c in range(NCORES):
        na = plan["node_at"][c]                    # [P, R] node ids or -1
        xp = np.zeros((P, R, IN_CH), dtype=np.float32)
        real = na >= 0
        xp[real] = x[na[real]]
        # xT layout: x_in[p_ch, h*NPADJ + r*P + p] = x[node_at(p,r), h*128+p_ch]
        xt = xp.reshape(P * R, IN_CH).T            # [256, (p r)]
        xt = xt.reshape(IN_CH, P, R).transpose(0, 2, 1).reshape(IN_CH, NPADJ)
        xt = xt.reshape(2, P, NPADJ).transpose(1, 0, 2).reshape(P, 2 * NPADJ)
        in_maps.append({
            "x": np.ascontiguousarray(xt).astype(bf16np),
            "w": W,
            "bias": bias,
            "emb": cluster_emb,
            "deg": plan["deg"][c],
            "idx": plan["idx"][c],
        })

    res = run_bass_kernel_spmd(nc, in_maps, core_ids=list(range(NCORES)))

    out = np.empty((N_NODES, IN_CH), dtype=np.float32)
    for c in range(NCORES):
        o = res.results[c]["out"].astype(np.float32)
        o = o.reshape(P, 2, NPADJ).transpose(1, 0, 2).reshape(IN_CH, NPADJ)
        na = plan["node_at"][c]                    # [P, R]
        j_of = (np.arange(R)[None, :].repeat(P, 0) * P
                + np.arange(P)[:, None].repeat(R, 1))
        real = na >= 0
        out[na[real]] = o[:, j_of[real]].T
    return out
